# revision 67
# baseline (speedup 1.0000x reference)
"""Trainium2 Bass kernel for LocalXLAttention (chunk-summed variant).

Math: the reference einsum sums over the chunk index z, so every query
attends to the same three [w, dh] K/V matrices built from chunk sums:
  K_prev = S_k - k_chunk[C-1], K_cur = S_k, K_next = S_k - k_chunk[0]
(and identically for V), where S_k = sum_c k_chunk[c].  Per position l
and head h:
  attn[l,h,:]  = qp[l,h,:] @ KbigT          (KbigT: [dh, 3w])
  probs        = softmax(attn, axis=-1)
  ctx[l,h,:]   = probs[l,h,:] @ Vbig        (Vbig:  [3w, dh])
  out          = ctx.reshape(L, dm) @ Wc
Sharding: L=4096 split 512 rows per core over 8 cores; each core
redundantly computes the tiny chunk-summed K/V from the full kv input.

Input loads use 1MB dma_start pieces: the ~2.5us fixed cost per
dma_start caps a ring at ~77GB/s with 256KB pieces but ~190GB/s with
1MB pieces, so the 11.25MB needed before the first QK lands in ~32us
(vs ~49us for the old 256KB layout).  kv pieces go first in d-order
(the chunk-sum tree consumes them in order), Wq/qT next, Wc during the
attention loop.

The attention loop is ACT-bound (exp at 1 elem/cycle/lane, ~1.1us per
[128,1024] step); QK pairs run concurrently on the PE's h0/h64 row
groups, PV + QP rides fill the rest of the PE slack.  Softmax
normalization stays on-chip: denominator rows -> reciprocal_approx_fast
(DVE) -> two K=1 PE matmuls broadcast 1/den across partitions -> DVE
multiply, placed a few steps after each pair so the PE never stalls on
the DVE chain.  (The previous version bounced denominators through DRAM
three times and cost ~20us of tail.)
"""

import sys
for _p in ('/opt/pypackages', '/opt/trn_rl_repo'):
    if _p not in sys.path:
        sys.path.insert(0, _p)

import numpy as np
import ml_dtypes

import concourse.bass as bass
import concourse.bacc as bacc
import concourse.tile as tile
from concourse import mybir
from concourse.bass_utils import run_bass_kernel_spmd
from concourse.masks import make_identity

F32 = mybir.dt.float32
BF16 = mybir.dt.bfloat16
AF = mybir.ActivationFunctionType

N_CORES = 8
L = 4096          # full sequence
LS = L // N_CORES # 512 rows per core
DM = 1024
NH = 16
DH = 64
W = 512           # chunk width
C = L // W        # 8 chunks
J3 = 3 * W        # 1536 softmax width
NJ = J3 // 128    # 12 j-chunks
DMT = DM // 128   # 8 dm-chunks
NPAIR = NH // 2   # 8 head pairs
NLT = LS // 128   # 4 output row chunks


def build_nc():
    nc = bacc.Bacc(None, target_bir_lowering=False)

    qT = nc.dram_tensor("qT", [DM, LS], BF16, kind="ExternalInput")
    kvT = nc.dram_tensor("kvT", [DM, L], BF16, kind="ExternalInput")
    Wq = nc.dram_tensor("Wq", [DM, DM], BF16, kind="ExternalInput")
    Wkv = nc.dram_tensor("Wkv", [DM, 2 * DH], BF16, kind="ExternalInput")
    Wc = nc.dram_tensor("Wc", [DM, DM], BF16, kind="ExternalInput")
    out = nc.dram_tensor("out", [LS, DM], BF16, kind="ExternalOutput")

    def chunk_src(dram, cols, d0, nd):
        # DRAM source AP delivering [128, nd, cols]: slot i holds rows
        # 128(d0+i):128(d0+i+1) of a [DM, cols] row-major tensor.
        base = dram[:, :]
        return bass.AP(tensor=base.tensor, offset=base.offset + d0 * 128 * cols,
                       ap=[[cols, 128], [cols * 128, nd], [1, cols]])

    with tile.TileContext(nc) as tc:
        with tc.tile_pool(name="weights", bufs=1) as wpool, \
             tc.tile_pool(name="small", bufs=1) as spool, \
             tc.tile_pool(name="qp", bufs=8) as qpool, \
             tc.tile_pool(name="stream", bufs=8) as stpool, \
             tc.tile_pool(name="kvsum", bufs=3) as kvspool, \
             tc.tile_pool(name="probs", bufs=2) as ppool, \
             tc.tile_pool(name="misc", bufs=2) as mpool, \
             tc.tile_pool(name="psacc", bufs=4, space="PSUM") as psacc, \
             tc.tile_pool(name="psmm", bufs=2, space="PSUM") as psmm:

            # ---------- input loads: 1MB pieces over the 3 DMA rings -----
            # kv first in d-order, Wq/qT next, Wc last (during the loop).
            wkv_sb = wpool.tile([128, DMT, 2 * DH], BF16, tag="wkv")
            nc.gpsimd.dma_start(out=wkv_sb, in_=chunk_src(Wkv, 2 * DH, 0, DMT))
            KV_ENG = (nc.sync, nc.scalar, nc.gpsimd, nc.sync,
                      nc.scalar, nc.gpsimd, nc.sync, nc.scalar)
            st_sb = []
            for d in range(DMT):
                st = stpool.tile([128, L], BF16, tag="kvstream", name=f"st{d}")
                st_sb.append(st)
                KV_ENG[d].dma_start(out=st,
                                    in_=kvT[128 * d:128 * (d + 1), :])
            # qT loads alongside kv; Wq arrives pair-major (host-permuted:
            # slab p holds Wq[:, 128p:128(p+1)] as [128 row-in-block, DMT,
            # 128 cols]) so only slab 0 is needed before the first QK - the
            # other slabs stream in during the attention loop, one pair
            # ahead of their QP rides.  Wc is needed only ~100us later: its
            # DMAs are emitted inside the kv-stream loop, gated on marker
            # copies (reading the last chunk sum, so the scheduler can't
            # hoist them) touching BOTH pieces' regions - the WAW dependency
            # keeps Wc out of the rings until the kv stream has drained.
            wq_sb = wpool.tile([128, NPAIR, DMT, 128], BF16, tag="wq")
            qt_sb = wpool.tile([128, DMT, LS], BF16, tag="qt")
            wc_sb = wpool.tile([128, DMT, DM], BF16, tag="wc")

            def wq_slab(p):
                base = Wq[:, :]
                return bass.AP(tensor=base.tensor,
                               offset=base.offset + p * 128 * DM,
                               ap=[[DM, 128], [1, DM]])

            nc.scalar.dma_start(out=wq_sb[:, 0, :, :], in_=wq_slab(0))
            nc.gpsimd.dma_start(out=wq_sb[:, 1, :, :], in_=wq_slab(1))
            nc.sync.dma_start(out=qt_sb, in_=chunk_src(qT, LS, 0, DMT))
            with tc.tile_wait_until(0.02):
                for p in range(2, NPAIR):
                    eng = (nc.scalar, nc.gpsimd, nc.sync)[p % 3]
                    eng.dma_start(out=wq_sb[:, p, :, :], in_=wq_slab(p))

            # ---------- constants ---------------------------------------
            zt = spool.tile([128, 512], BF16, tag="zt")
            nc.vector.memset(zt, 0.0)
            ident = spool.tile([128, 128], BF16, tag="ident")
            make_identity(nc, ident)
            o64 = spool.tile([1, DH], BF16, tag="o64")
            nc.vector.memset(o64, 1.0)
            ones_sb = spool.tile([128, 1], BF16, tag="ones")
            nc.vector.memset(ones_sb, 1.0)

            # ---------- PE warm-up during the DMA wait (HAM clock gate) --
            warm_ps = psacc.tile([128, W], F32, tag="acc", name="warm")
            for i in range(36):
                nc.tensor.matmul(warm_ps[:, 0:128], zt[:, 0:128], zt[:, 0:128],
                                 start=True, stop=True)
            # preload the exp activation table during startup
            exp_warm = spool.tile([1, 8], F32, tag="expwarm")
            nc.scalar.activation(exp_warm, zt[0:1, 0:8], AF.Exp, scale=1.0)

            # ---------- QP_T machinery (pair 0 runs mid-kv-stream) -------
            qpt_sb = [None] * NPAIR

            def emit_qp_mm(ps, t, d):
                nc.tensor.matmul(ps, wq_sb[:, t, d, :],
                                 qt_sb[:, d, :],
                                 start=(d == 0), stop=(d == DMT - 1))

            def emit_qp(t):
                ps = psacc.tile([128, W], F32, tag="acc", name=f"qps{t}")
                for d in range(DMT):
                    emit_qp_mm(ps, t, d)
                sb = qpool.tile([128, LS], BF16, tag="qpt", name=f"qpt{t}")
                nc.vector.tensor_copy(sb, ps)
                qpt_sb[t] = sb

            # ---------- kv stream: chunk-sum tree + projections ----------
            # PSUM accumulators pack K rows 0:64, V rows 64:128.
            ps0 = psacc.tile([128, W], F32, tag="acc", name="ps0")
            ps7 = psacc.tile([128, W], F32, tag="acc", name="ps7")
            pss = psacc.tile([128, W], F32, tag="acc", name="pss")
            for d in range(DMT):
                st = st_sb[d]
                nc.tensor.matmul(ps0, wkv_sb[:, d, :], st[:, 0:W],
                                 start=(d == 0), stop=(d == DMT - 1))
                nc.tensor.matmul(ps7, wkv_sb[:, d, :], st[:, L - W:L],
                                 start=(d == 0), stop=(d == DMT - 1))
                nc.vector.tensor_add(st[:, 0:2048], st[:, 0:2048], st[:, 2048:4096])
                nc.vector.tensor_add(st[:, 0:1024], st[:, 0:1024], st[:, 1024:2048])
                ks = kvspool.tile([128, W], BF16, tag="kvsum")
                nc.vector.tensor_add(ks, st[:, 0:512], st[:, 512:1024])
                nc.tensor.matmul(pss, wkv_sb[:, d, :], ks,
                                 start=(d == 0), stop=(d == DMT - 1))
                if d == 3:
                    # pair 0's QP_T runs here, in the PE's idle windows
                    # between kv piece arrivals (qT and Wq slab 0 landed
                    # long ago); its result is then ready the moment Kbig
                    # is, instead of serializing after the kv tree.
                    emit_qp(0)
                if d == DMT - 1:
                    nc.vector.tensor_copy(wc_sb[0:1, 0, 0:1], ks[0:1, 0:1])
                    nc.vector.tensor_copy(wc_sb[0:1, 4, 0:1], ks[0:1, 0:1])
                    nc.sync.dma_start(out=wc_sb[:, 0:4, :],
                                      in_=chunk_src(Wc, DM, 0, 4))
                    nc.gpsimd.dma_start(out=wc_sb[:, 4:8, :],
                                        in_=chunk_src(Wc, DM, 4, 4))

            # ---------- evacuate K/V variants to SBUF (bf16) -------------
            kv0_sb = spool.tile([128, W], BF16, tag="kv0")  # K rows 0:64, V 64:128
            kv7_sb = spool.tile([128, W], BF16, tag="kv7")
            kvs_sb = spool.tile([128, W], BF16, tag="kvs")
            nc.vector.tensor_copy(kv0_sb, ps0)
            nc.vector.tensor_copy(kv7_sb, ps7)
            nc.vector.tensor_copy(kvs_sb, pss)

            # ---------- KbigT [128, 1536] = [prev | cur | next] ----------
            # rows 64:128 duplicate rows 0:64 so the QK pair can run on
            # both PE row groups concurrently.
            kbig = spool.tile([128, J3], BF16, tag="kbig")
            nc.vector.tensor_sub(kbig[0:DH, 0:W], kvs_sb[0:DH, :], kv7_sb[0:DH, :])
            nc.vector.tensor_copy(kbig[0:DH, W:2 * W], kvs_sb[0:DH, :])
            nc.vector.tensor_sub(kbig[0:DH, 2 * W:3 * W], kvs_sb[0:DH, :],
                                 kv0_sb[0:DH, :])
            nc.vector.tensor_copy(kbig[DH:2 * DH, :], kbig[0:DH, :])

            # ---------- Vbig [128, 12, 65(+pad)] -------------------------
            vbig = spool.tile([128, NJ, 68], BF16, tag="vbig")
            for j in range(NJ):
                nc.vector.tensor_copy(vbig[:, j, DH:DH + 1], ones_sb)
            for yt in range(4):
                tps = psacc.tile([128, DH], BF16, tag="acc")
                tp0 = psacc.tile([128, DH], BF16, tag="acc")
                tp7 = psacc.tile([128, DH], BF16, tag="acc")
                sl = slice(128 * yt, 128 * (yt + 1))
                # V rows live at base partition 64; ident[64:128, 64:128]
                # is an identity block at the matching base.
                idq = ident[DH:128, DH:128]
                nc.tensor.transpose(tps, kvs_sb[DH:128, sl], idq)
                nc.tensor.transpose(tp0, kv0_sb[DH:128, sl], idq)
                nc.tensor.transpose(tp7, kv7_sb[DH:128, sl], idq)
                nc.vector.tensor_copy(vbig[:, 4 + yt, 0:DH], tps)
                nc.vector.tensor_sub(vbig[:, 0 + yt, 0:DH], vbig[:, 4 + yt, 0:DH], tp7)
                nc.vector.tensor_sub(vbig[:, 8 + yt, 0:DH], vbig[:, 4 + yt, 0:DH], tp0)

            # a few warm matmuls keep the PE clock ramped through the DVE
            # kbig/vbig construction so the first QKs don't run cold (a
            # fresh psmm tile: warm_ps's psacc slot was recycled long ago)
            warm2 = psmm.tile([128, 1024], F32, tag="mm", name="warm2")
            for i in range(10):
                nc.tensor.matmul(warm2[:, 0:128], zt[:, 0:128], zt[:, 0:128],
                                 start=True, stop=True)

            # ---------- attention: QK -> exp(PSUM direct) -> PV ----------
            ctxu_sb = []  # per pair [128, 512]: rows 0:64 head 2t, 64:128 head 2t+1
            for t in range(NPAIR):
                ctxu_sb.append(qpool.tile([128, W], BF16, tag="ctxu",
                                          name=f"ctxu{t}"))

            steps = [(t, j) for t in range(NPAIR) for j in range(NJ)]
            ctx_ps = {}   # t -> (ctxA, ctxB)
            qps_ps = {}   # t -> psum tile being accumulated
            qk_tiles = {}
            rdb_sb = {}   # t -> [1, 2W] bf16 reciprocal denominators
            wc_ride = {}  # psum tile for the (lt=2, half=0) Wc ride

            def emit_qk(t, j):
                qpt = qpt_sb[t]
                qk = psmm.tile([128, 1024], F32, tag="mm", name=f"qk{t}_{j}")
                nc.tensor.matmul(qk[:, 0:W],
                                 kbig[0:DH, 128 * j:128 * (j + 1)],
                                 qpt[0:DH, :], start=True, stop=True)
                nc.tensor.matmul(qk[:, W:2 * W],
                                 kbig[DH:2 * DH, 128 * j:128 * (j + 1)],
                                 qpt[DH:128, :], start=True, stop=True)
                qk_tiles[(t, j)] = qk

            def finish_pair(t):
                # Evacuate unnormalized context + the denominator rows,
                # then build 1/den while the next pair streams.  The PE
                # broadcast + DVE multiply run a few steps later
                # (norm_tail) so the PE never waits on this DVE chain.
                ctxA, ctxB = ctx_ps.pop(t)
                cu = ctxu_sb[t]
                dd = mpool.tile([1, 2 * W], F32, tag="dd", name=f"dd{t}", bufs=2)
                nc.vector.tensor_copy(cu[0:DH, :], ctxA[0:DH, :])
                nc.vector.tensor_copy(dd[0:1, 0:W], ctxA[DH:DH + 1, :])
                nc.vector.tensor_copy(cu[DH:128, :], ctxB[0:DH, :])
                nc.vector.tensor_copy(dd[0:1, W:2 * W], ctxB[DH:DH + 1, :])
                rd = mpool.tile([1, 2 * W], F32, tag="rd", name=f"rd{t}", bufs=2)
                nc.vector.reciprocal_approx_fast(out=rd, in_=dd)
                rdb = mpool.tile([1, 2 * W], BF16, tag="rdb", name=f"rdb{t}",
                                 bufs=2)
                nc.vector.tensor_copy(rdb, rd)
                rdb_sb[t] = rdb

            bcp_ps = {}

            def norm_tail_a(t):
                # broadcast 1/den across partitions with two K=1 PE matmuls
                # (rows 0:64 <- head 2t, rows 64:128 <- head 2t+1), then
                # scale ctxu in place.  Split in two halves (called at
                # different steps) so the inserted PE work never exceeds the
                # per-step slack behind ACT.
                bcp = psacc.tile([128, W], F32, tag="acc", name=f"bcp{t}")
                bcp_ps[t] = bcp
                nc.tensor.matmul(bcp[0:DH, :], o64, rdb_sb[t][0:1, 0:W],
                                 start=True, stop=True)
                nc.vector.tensor_mul(ctxu_sb[t][0:DH, :],
                                     ctxu_sb[t][0:DH, :], bcp[0:DH, :])

            def norm_tail_b(t):
                rdb = rdb_sb.pop(t)
                bcp = bcp_ps.pop(t)
                nc.tensor.matmul(bcp[DH:128, :], o64, rdb[0:1, W:2 * W],
                                 start=True, stop=True)
                nc.vector.tensor_mul(ctxu_sb[t][DH:128, :],
                                     ctxu_sb[t][DH:128, :], bcp[DH:128, :])

            def norm_tail(t):
                norm_tail_a(t)
                norm_tail_b(t)

            emit_qk(*steps[0])
            for s in range(len(steps)):
                t, j = steps[s]
                if s + 1 < len(steps):
                    emit_qk(*steps[s + 1])
                if j == 0:
                    ctxA = psacc.tile([128, W], F32, tag="acc", name=f"ctxA{t}")
                    ctxB = psacc.tile([128, W], F32, tag="acc", name=f"ctxB{t}")
                    ctx_ps[t] = (ctxA, ctxB)
                    if t + 1 < NPAIR:
                        qps_ps[t + 1] = psacc.tile([128, W], F32, tag="acc",
                                                   name=f"qps{t + 1}")
                ctxA, ctxB = ctx_ps[t]
                qk = qk_tiles.pop((t, j))
                pr = ppool.tile([128, 1024], BF16, tag="probs",
                                name=f"pr{t}_{j}")
                nc.scalar.activation(pr, qk, AF.Exp, scale=0.125)
                nc.tensor.matmul(ctxA[0:DH + 1, :], vbig[:, j, 0:DH + 1],
                                 pr[:, 0:W],
                                 start=(j == 0), stop=(j == NJ - 1))
                nc.tensor.matmul(ctxB[0:DH + 1, :], vbig[:, j, 0:DH + 1],
                                 pr[:, W:2 * W],
                                 start=(j == 0), stop=(j == NJ - 1))
                # ride the next pair's QP_T matmuls in ACT's slack (the
                # j-2 shift leaves time for the pair's Wq slab to land),
                # then evacuate at j==10 so QK(t+1, 0) finds it ready.
                if t + 1 in qps_ps and 2 <= j < DMT + 2:
                    emit_qp_mm(qps_ps[t + 1], t + 1, j - 2)
                # the last pair has no QP ride, freeing one psacc bank:
                # ride (lt=2, half=0) of the output projection there
                # (he 0..5 only: he6 is normalized at (7,11), too late)
                if t == NPAIR - 1 and 1 <= j <= DMT - 2:
                    he = j - 1
                    if he == 0:
                        wc_ride[0] = psacc.tile([128, W], F32, tag="acc",
                                                name="wcr2_0")
                    nc.tensor.matmul(wc_ride[0],
                                     ctxu_sb[he][:, 256:384],
                                     wc_sb[:, he, 0:512],
                                     start=(he == 0), stop=False)
                if j == DMT + 2 and t + 1 in qps_ps:
                    qps = qps_ps.pop(t + 1)
                    sb = qpool.tile([128, LS], BF16, tag="qpt",
                                    name=f"qpt{t + 1}")
                    nc.vector.tensor_copy(sb, qps)
                    qpt_sb[t + 1] = sb
                # normalize pair t-1 in the ride-free steps j=10/11, where
                # the PE has ~450ns of slack behind ACT (at j=4/6 the bcp
                # matmuls overran the slack and rippled into the exp stream)
                if j == 10 and t > 0:
                    norm_tail_a(t - 1)
                if j == 11 and t > 0:
                    norm_tail_b(t - 1)
                if j == NJ - 1:
                    finish_pair(t)

            # ---------- out = ctx @ Wc, evacuate, store ----------
            # All (lt, half) groups accumulate he 0..6 first (overlapping
            # the last pair's normalization chain), then each group's he=7
            # lands and the result evacuates + stores, pipelined per lt.
            wc_halves = {}

            def emit_wc(lt, he_list):
                for half in range(2):
                    for he in he_list:
                        nc.tensor.matmul(
                            wc_halves[lt][half],
                            ctxu_sb[he][:, 128 * lt:128 * (lt + 1)],
                            wc_sb[:, he, 512 * half:512 * (half + 1)],
                            start=(he == 0), stop=(he == DMT - 1))

            for lt in (0, 1):
                wcp = psmm.tile([128, 1024], F32, tag="mm", name=f"wcp{lt}")
                wc_halves[lt] = (wcp[:, 0:512], wcp[:, 512:1024])
                emit_wc(lt, range(7))
            # the last pair's normalization lands here: its DVE reciprocal
            # chain overlaps the lt0/lt1 matmuls above so the bcp matmuls
            # don't stall the PE, and the psacc pool still has a free slot
            # (lt2/lt3 accumulators are allocated after).
            norm_tail(NPAIR - 1)
            # (lt=2, half=0) accumulated he 0..5 inside pair 7 (wc_ride);
            # its he=6 lands here, half=1 runs he 0..6 from scratch
            wc_halves[2] = (wc_ride[0],
                            psacc.tile([128, W], F32, tag="acc", name="wcp2_1"))
            nc.tensor.matmul(wc_halves[2][0], ctxu_sb[6][:, 256:384],
                             wc_sb[:, 6, 0:512], start=False, stop=False)
            for he in range(7):
                nc.tensor.matmul(wc_halves[2][1],
                                 ctxu_sb[he][:, 256:384],
                                 wc_sb[:, he, 512:1024],
                                 start=(he == 0), stop=False)
            wc_halves[3] = (psacc.tile([128, W], F32, tag="acc", name="wcp3_0"),
                            psacc.tile([128, W], F32, tag="acc", name="wcp3_1"))
            emit_wc(3, range(7))
            OUT_ENG = (nc.sync, nc.scalar, nc.gpsimd, nc.sync)
            for lt in range(NLT):
                emit_wc(lt, [7])
                halves = wc_halves[lt]
                ob = mpool.tile([128, DM], BF16, tag="outsb", bufs=2)
                if lt % 2 == 0:
                    nc.scalar.activation(ob[:, 0:512], halves[0], AF.Copy)
                    nc.scalar.activation(ob[:, 512:1024], halves[1], AF.Copy)
                else:
                    nc.vector.tensor_copy(ob[:, 0:512], halves[0])
                    nc.vector.tensor_copy(ob[:, 512:1024], halves[1])
                OUT_ENG[lt].dma_start(out=out[128 * lt:128 * (lt + 1), :],
                                      in_=ob)

    nc.compile()
    return nc


_NC = None


def _get_nc():
    global _NC
    if _NC is None:
        _NC = build_nc()
    return _NC


def prep_in_maps(q, kv, Wq, Wkv, Wc):
    """Host-side input prep: transpose, cast to bf16, shard queries."""
    bf16 = ml_dtypes.bfloat16
    qT_full = np.ascontiguousarray(np.asarray(q, dtype=np.float32)[0].T
                                   ).astype(bf16)
    kvT = np.ascontiguousarray(np.asarray(kv, dtype=np.float32)[0].T
                               ).astype(bf16)
    # pair-major Wq: slab p = Wq[:, 128p:128(p+1)] laid out as
    # [row-in-block(128), d-block(8), col(128)], contiguous per slab
    Wq = np.asarray(Wq, dtype=np.float32).reshape(DMT, 128, NPAIR, 128)
    Wq = np.ascontiguousarray(Wq.transpose(2, 1, 0, 3).reshape(DM, DM)
                              ).astype(bf16)
    Wkv = np.ascontiguousarray(np.asarray(Wkv, dtype=np.float32)).astype(bf16)
    Wc = np.ascontiguousarray(np.asarray(Wc, dtype=np.float32)).astype(bf16)
    in_maps = []
    for i in range(N_CORES):
        in_maps.append({
            "qT": np.ascontiguousarray(qT_full[:, LS * i:LS * (i + 1)]),
            "kvT": kvT,
            "Wq": Wq,
            "Wkv": Wkv,
            "Wc": Wc,
        })
    return in_maps


def kernel(q, kv, Wq, Wkv, Wc, w):
    assert int(w) == W
    q = np.asarray(q, dtype=np.float32)
    B = q.shape[0]
    assert B == 1 and q.shape[1] == L and q.shape[2] == DM

    in_maps = prep_in_maps(q, kv, Wq, Wkv, Wc)
    nc = _get_nc()
    res = run_bass_kernel_spmd(nc, in_maps, list(range(N_CORES)))
    out = np.concatenate([res.results[i]["out"] for i in range(N_CORES)], axis=0)
    return out.reshape(1, L, DM).astype(np.float32)


# revision 69
# speedup vs baseline: 1.0036x; 1.0036x over previous
"""Trainium2 Bass kernel for LocalXLAttention (chunk-summed variant).

Math: the reference einsum sums over the chunk index z, so every query
attends to the same three [w, dh] K/V matrices built from chunk sums:
  K_prev = S_k - k_chunk[C-1], K_cur = S_k, K_next = S_k - k_chunk[0]
(and identically for V), where S_k = sum_c k_chunk[c].  Per position l
and head h:
  attn[l,h,:]  = qp[l,h,:] @ KbigT          (KbigT: [dh, 3w])
  probs        = softmax(attn, axis=-1)
  ctx[l,h,:]   = probs[l,h,:] @ Vbig        (Vbig:  [3w, dh])
  out          = ctx.reshape(L, dm) @ Wc
Sharding: L=4096 split 512 rows per core over 8 cores; each core
redundantly computes the tiny chunk-summed K/V from the full kv input.

Input loads use 1MB dma_start pieces: the ~2.5us fixed cost per
dma_start caps a ring at ~77GB/s with 256KB pieces but ~190GB/s with
1MB pieces, so the 11.25MB needed before the first QK lands in ~32us
(vs ~49us for the old 256KB layout).  kv pieces go first in d-order
(the chunk-sum tree consumes them in order), Wq/qT next, Wc during the
attention loop.

The attention loop is ACT-bound (exp at 1 elem/cycle/lane, ~1.1us per
[128,1024] step); QK pairs run concurrently on the PE's h0/h64 row
groups, PV + QP rides fill the rest of the PE slack.  Softmax
normalization stays on-chip: denominator rows -> reciprocal_approx_fast
(DVE) -> two K=1 PE matmuls broadcast 1/den across partitions -> DVE
multiply, placed a few steps after each pair so the PE never stalls on
the DVE chain.  (The previous version bounced denominators through DRAM
three times and cost ~20us of tail.)
"""

import sys
for _p in ('/opt/pypackages', '/opt/trn_rl_repo'):
    if _p not in sys.path:
        sys.path.insert(0, _p)

import numpy as np
import ml_dtypes

import concourse.bass as bass
import concourse.bacc as bacc
import concourse.tile as tile
from concourse import mybir
from concourse.bass_utils import run_bass_kernel_spmd
from concourse.masks import make_identity

F32 = mybir.dt.float32
BF16 = mybir.dt.bfloat16
AF = mybir.ActivationFunctionType

N_CORES = 8
L = 4096          # full sequence
LS = L // N_CORES # 512 rows per core
DM = 1024
NH = 16
DH = 64
W = 512           # chunk width
C = L // W        # 8 chunks
J3 = 3 * W        # 1536 softmax width
NJ = J3 // 128    # 12 j-chunks
DMT = DM // 128   # 8 dm-chunks
NPAIR = NH // 2   # 8 head pairs
NLT = LS // 128   # 4 output row chunks


def build_nc():
    nc = bacc.Bacc(None, target_bir_lowering=False)

    qT = nc.dram_tensor("qT", [DM, LS], BF16, kind="ExternalInput")
    kvT = nc.dram_tensor("kvT", [DM, L], BF16, kind="ExternalInput")
    Wq = nc.dram_tensor("Wq", [DM, DM], BF16, kind="ExternalInput")
    Wkv = nc.dram_tensor("Wkv", [DM, 2 * DH], BF16, kind="ExternalInput")
    Wc = nc.dram_tensor("Wc", [DM, DM], BF16, kind="ExternalInput")
    out = nc.dram_tensor("out", [LS, DM], BF16, kind="ExternalOutput")

    def chunk_src(dram, cols, d0, nd):
        # DRAM source AP delivering [128, nd, cols]: slot i holds rows
        # 128(d0+i):128(d0+i+1) of a [DM, cols] row-major tensor.
        base = dram[:, :]
        return bass.AP(tensor=base.tensor, offset=base.offset + d0 * 128 * cols,
                       ap=[[cols, 128], [cols * 128, nd], [1, cols]])

    with tile.TileContext(nc) as tc:
        with tc.tile_pool(name="weights", bufs=1) as wpool, \
             tc.tile_pool(name="small", bufs=1) as spool, \
             tc.tile_pool(name="qp", bufs=8) as qpool, \
             tc.tile_pool(name="stream", bufs=8) as stpool, \
             tc.tile_pool(name="kvsum", bufs=3) as kvspool, \
             tc.tile_pool(name="probs", bufs=2) as ppool, \
             tc.tile_pool(name="misc", bufs=2) as mpool, \
             tc.tile_pool(name="psacc", bufs=4, space="PSUM") as psacc, \
             tc.tile_pool(name="psmm", bufs=2, space="PSUM") as psmm:

            # ---------- input loads: 1MB pieces over the 3 DMA rings -----
            # kv first in d-order, Wq/qT next, Wc last (during the loop).
            wkv_sb = wpool.tile([128, DMT, 2 * DH], BF16, tag="wkv")
            nc.gpsimd.dma_start(out=wkv_sb, in_=chunk_src(Wkv, 2 * DH, 0, DMT))
            KV_ENG = (nc.sync, nc.scalar, nc.gpsimd, nc.sync,
                      nc.scalar, nc.gpsimd, nc.sync, nc.scalar)
            st_sb = []
            for d in range(DMT):
                st = stpool.tile([128, L], BF16, tag="kvstream", name=f"st{d}")
                st_sb.append(st)
                KV_ENG[d].dma_start(out=st,
                                    in_=kvT[128 * d:128 * (d + 1), :])
            # qT loads alongside kv; Wq arrives pair-major (host-permuted:
            # slab p holds Wq[:, 128p:128(p+1)] as [128 row-in-block, DMT,
            # 128 cols]) so only slab 0 is needed before the first QK - the
            # other slabs stream in during the attention loop, one pair
            # ahead of their QP rides.  Wc is needed only ~100us later: its
            # DMAs are emitted inside the kv-stream loop, gated on marker
            # copies (reading the last chunk sum, so the scheduler can't
            # hoist them) touching BOTH pieces' regions - the WAW dependency
            # keeps Wc out of the rings until the kv stream has drained.
            wq_sb = wpool.tile([128, NPAIR, DMT, 128], BF16, tag="wq")
            qt_sb = wpool.tile([128, DMT, LS], BF16, tag="qt")
            wc_sb = wpool.tile([128, DMT, DM], BF16, tag="wc")

            def wq_slab(p):
                base = Wq[:, :]
                return bass.AP(tensor=base.tensor,
                               offset=base.offset + p * 128 * DM,
                               ap=[[DM, 128], [1, DM]])

            nc.scalar.dma_start(out=wq_sb[:, 0, :, :], in_=wq_slab(0))
            nc.gpsimd.dma_start(out=wq_sb[:, 1, :, :], in_=wq_slab(1))
            nc.sync.dma_start(out=qt_sb, in_=chunk_src(qT, LS, 0, DMT))
            with tc.tile_wait_until(0.02):
                for p in range(2, NPAIR):
                    eng = (nc.scalar, nc.gpsimd, nc.sync)[p % 3]
                    eng.dma_start(out=wq_sb[:, p, :, :], in_=wq_slab(p))

            # ---------- constants ---------------------------------------
            zt = spool.tile([128, 512], BF16, tag="zt")
            nc.vector.memset(zt, 0.0)
            ident = spool.tile([128, 128], BF16, tag="ident")
            make_identity(nc, ident)
            o64 = spool.tile([1, DH], BF16, tag="o64")
            nc.vector.memset(o64, 1.0)
            ones_sb = spool.tile([128, 1], BF16, tag="ones")
            nc.vector.memset(ones_sb, 1.0)

            # ---------- PE warm-up during the DMA wait (HAM clock gate) --
            warm_ps = psacc.tile([128, W], F32, tag="acc", name="warm")
            for i in range(36):
                nc.tensor.matmul(warm_ps[:, 0:128], zt[:, 0:128], zt[:, 0:128],
                                 start=True, stop=True)
            # preload the exp activation table during startup
            exp_warm = spool.tile([1, 8], F32, tag="expwarm")
            nc.scalar.activation(exp_warm, zt[0:1, 0:8], AF.Exp, scale=1.0)

            # ---------- QP_T machinery (pair 0 runs mid-kv-stream) -------
            qpt_sb = [None] * NPAIR

            def emit_qp_mm(ps, t, d):
                nc.tensor.matmul(ps, wq_sb[:, t, d, :],
                                 qt_sb[:, d, :],
                                 start=(d == 0), stop=(d == DMT - 1))

            def emit_qp(t):
                ps = psacc.tile([128, W], F32, tag="acc", name=f"qps{t}")
                for d in range(DMT):
                    emit_qp_mm(ps, t, d)
                sb = qpool.tile([128, LS], BF16, tag="qpt", name=f"qpt{t}")
                nc.vector.tensor_copy(sb, ps)
                qpt_sb[t] = sb

            # ---------- kv stream: chunk-sum tree + projections ----------
            # PSUM accumulators pack K rows 0:64, V rows 64:128.
            ps0 = psacc.tile([128, W], F32, tag="acc", name="ps0")
            ps7 = psacc.tile([128, W], F32, tag="acc", name="ps7")
            pss = psacc.tile([128, W], F32, tag="acc", name="pss")
            for d in range(DMT):
                st = st_sb[d]
                nc.tensor.matmul(ps0, wkv_sb[:, d, :], st[:, 0:W],
                                 start=(d == 0), stop=(d == DMT - 1))
                nc.tensor.matmul(ps7, wkv_sb[:, d, :], st[:, L - W:L],
                                 start=(d == 0), stop=(d == DMT - 1))
                nc.vector.tensor_add(st[:, 0:2048], st[:, 0:2048], st[:, 2048:4096])
                nc.vector.tensor_add(st[:, 0:1024], st[:, 0:1024], st[:, 1024:2048])
                ks = kvspool.tile([128, W], BF16, tag="kvsum")
                nc.vector.tensor_add(ks, st[:, 0:512], st[:, 512:1024])
                nc.tensor.matmul(pss, wkv_sb[:, d, :], ks,
                                 start=(d == 0), stop=(d == DMT - 1))
                if d == 3:
                    # pair 0's QP_T runs here, in the PE's idle windows
                    # between kv piece arrivals (qT and Wq slab 0 landed
                    # long ago); its result is then ready the moment Kbig
                    # is, instead of serializing after the kv tree.
                    emit_qp(0)
                if d == DMT - 1:
                    nc.vector.tensor_copy(wc_sb[0:1, 0, 0:1], ks[0:1, 0:1])
                    nc.vector.tensor_copy(wc_sb[0:1, 4, 0:1], ks[0:1, 0:1])
                    nc.sync.dma_start(out=wc_sb[:, 0:4, :],
                                      in_=chunk_src(Wc, DM, 0, 4))
                    nc.gpsimd.dma_start(out=wc_sb[:, 4:8, :],
                                        in_=chunk_src(Wc, DM, 4, 4))

            # ---------- evacuate K/V variants to SBUF (bf16) -------------
            # interleaved with the KbigT build in j-consumption order so
            # the first QK fires ~1.2us sooner: the 'prev' block (+ its
            # row-dup, j=0..3) needs only kvs/kv7, before kv0 evacuates.
            kv0_sb = spool.tile([128, W], BF16, tag="kv0")  # K rows 0:64, V 64:128
            kv7_sb = spool.tile([128, W], BF16, tag="kv7")
            kvs_sb = spool.tile([128, W], BF16, tag="kvs")
            kbig = spool.tile([128, J3], BF16, tag="kbig")
            # KbigT [128, 1536] = [prev | cur | next]; rows 64:128
            # duplicate rows 0:64 so the QK pair can run on both PE row
            # groups concurrently.
            nc.vector.tensor_copy(kvs_sb, pss)
            nc.vector.tensor_copy(kv7_sb, ps7)
            nc.vector.tensor_sub(kbig[0:DH, 0:W], kvs_sb[0:DH, :], kv7_sb[0:DH, :])
            nc.vector.tensor_copy(kbig[DH:2 * DH, 0:W], kbig[0:DH, 0:W])
            nc.vector.tensor_copy(kbig[0:DH, W:2 * W], kvs_sb[0:DH, :])
            nc.vector.tensor_copy(kbig[DH:2 * DH, W:2 * W], kvs_sb[0:DH, :])
            nc.vector.tensor_copy(kv0_sb, ps0)
            nc.vector.tensor_sub(kbig[0:DH, 2 * W:3 * W], kvs_sb[0:DH, :],
                                 kv0_sb[0:DH, :])
            nc.vector.tensor_copy(kbig[DH:2 * DH, 2 * W:3 * W],
                                  kbig[0:DH, 2 * W:3 * W])

            # ---------- Vbig [128, 12, 65(+pad)] -------------------------
            vbig = spool.tile([128, NJ, 68], BF16, tag="vbig")
            for j in range(NJ):
                nc.vector.tensor_copy(vbig[:, j, DH:DH + 1], ones_sb)
            for yt in range(4):
                tps = psacc.tile([128, DH], BF16, tag="acc")
                tp0 = psacc.tile([128, DH], BF16, tag="acc")
                tp7 = psacc.tile([128, DH], BF16, tag="acc")
                sl = slice(128 * yt, 128 * (yt + 1))
                # V rows live at base partition 64; ident[64:128, 64:128]
                # is an identity block at the matching base.
                idq = ident[DH:128, DH:128]
                nc.tensor.transpose(tps, kvs_sb[DH:128, sl], idq)
                nc.tensor.transpose(tp0, kv0_sb[DH:128, sl], idq)
                nc.tensor.transpose(tp7, kv7_sb[DH:128, sl], idq)
                nc.vector.tensor_copy(vbig[:, 4 + yt, 0:DH], tps)
                nc.vector.tensor_sub(vbig[:, 0 + yt, 0:DH], vbig[:, 4 + yt, 0:DH], tp7)
                nc.vector.tensor_sub(vbig[:, 8 + yt, 0:DH], vbig[:, 4 + yt, 0:DH], tp0)

            # a few warm matmuls keep the PE clock ramped through the DVE
            # kbig/vbig construction so the first QKs don't run cold (a
            # fresh psmm tile: warm_ps's psacc slot was recycled long ago)
            warm2 = psmm.tile([128, 1024], F32, tag="mm", name="warm2")
            for i in range(10):
                nc.tensor.matmul(warm2[:, 0:128], zt[:, 0:128], zt[:, 0:128],
                                 start=True, stop=True)

            # ---------- attention: QK -> exp(PSUM direct) -> PV ----------
            ctxu_sb = []  # per pair [128, 512]: rows 0:64 head 2t, 64:128 head 2t+1
            for t in range(NPAIR):
                ctxu_sb.append(qpool.tile([128, W], BF16, tag="ctxu",
                                          name=f"ctxu{t}"))

            steps = [(t, j) for t in range(NPAIR) for j in range(NJ)]
            ctx_ps = {}   # t -> (ctxA, ctxB)
            qps_ps = {}   # t -> psum tile being accumulated
            qk_tiles = {}
            rdb_sb = {}   # t -> [1, 2W] bf16 reciprocal denominators
            wc_ride = {}  # psum tile for the (lt=2, half=0) Wc ride

            def emit_qk(t, j):
                qpt = qpt_sb[t]
                qk = psmm.tile([128, 1024], F32, tag="mm", name=f"qk{t}_{j}")
                nc.tensor.matmul(qk[:, 0:W],
                                 kbig[0:DH, 128 * j:128 * (j + 1)],
                                 qpt[0:DH, :], start=True, stop=True)
                nc.tensor.matmul(qk[:, W:2 * W],
                                 kbig[DH:2 * DH, 128 * j:128 * (j + 1)],
                                 qpt[DH:128, :], start=True, stop=True)
                qk_tiles[(t, j)] = qk

            def finish_pair(t):
                # Evacuate unnormalized context + the denominator rows,
                # then build 1/den while the next pair streams.  The PE
                # broadcast + DVE multiply run a few steps later
                # (norm_tail) so the PE never waits on this DVE chain.
                ctxA, ctxB = ctx_ps.pop(t)
                cu = ctxu_sb[t]
                dd = mpool.tile([1, 2 * W], F32, tag="dd", name=f"dd{t}", bufs=2)
                nc.vector.tensor_copy(cu[0:DH, :], ctxA[0:DH, :])
                nc.vector.tensor_copy(dd[0:1, 0:W], ctxA[DH:DH + 1, :])
                nc.vector.tensor_copy(cu[DH:128, :], ctxB[0:DH, :])
                nc.vector.tensor_copy(dd[0:1, W:2 * W], ctxB[DH:DH + 1, :])
                rd = mpool.tile([1, 2 * W], F32, tag="rd", name=f"rd{t}", bufs=2)
                nc.vector.reciprocal_approx_fast(out=rd, in_=dd)
                rdb = mpool.tile([1, 2 * W], BF16, tag="rdb", name=f"rdb{t}",
                                 bufs=2)
                nc.vector.tensor_copy(rdb, rd)
                rdb_sb[t] = rdb

            bcp_ps = {}

            def norm_tail_a(t):
                # broadcast 1/den across partitions with two K=1 PE matmuls
                # (rows 0:64 <- head 2t, rows 64:128 <- head 2t+1), then
                # scale ctxu in place.  Split in two halves (called at
                # different steps) so the inserted PE work never exceeds the
                # per-step slack behind ACT.
                bcp = psacc.tile([128, W], F32, tag="acc", name=f"bcp{t}")
                bcp_ps[t] = bcp
                nc.tensor.matmul(bcp[0:DH, :], o64, rdb_sb[t][0:1, 0:W],
                                 start=True, stop=True)
                nc.vector.tensor_mul(ctxu_sb[t][0:DH, :],
                                     ctxu_sb[t][0:DH, :], bcp[0:DH, :])

            def norm_tail_b(t):
                rdb = rdb_sb.pop(t)
                bcp = bcp_ps.pop(t)
                nc.tensor.matmul(bcp[DH:128, :], o64, rdb[0:1, W:2 * W],
                                 start=True, stop=True)
                nc.vector.tensor_mul(ctxu_sb[t][DH:128, :],
                                     ctxu_sb[t][DH:128, :], bcp[DH:128, :])

            def norm_tail(t):
                norm_tail_a(t)
                norm_tail_b(t)

            emit_qk(*steps[0])
            for s in range(len(steps)):
                t, j = steps[s]
                if s + 1 < len(steps):
                    emit_qk(*steps[s + 1])
                if j == 0:
                    ctxA = psacc.tile([128, W], F32, tag="acc", name=f"ctxA{t}")
                    ctxB = psacc.tile([128, W], F32, tag="acc", name=f"ctxB{t}")
                    ctx_ps[t] = (ctxA, ctxB)
                    if t + 1 < NPAIR:
                        qps_ps[t + 1] = psacc.tile([128, W], F32, tag="acc",
                                                   name=f"qps{t + 1}")
                ctxA, ctxB = ctx_ps[t]
                qk = qk_tiles.pop((t, j))
                pr = ppool.tile([128, 1024], BF16, tag="probs",
                                name=f"pr{t}_{j}")
                nc.scalar.activation(pr, qk, AF.Exp, scale=0.125)
                nc.tensor.matmul(ctxA[0:DH + 1, :], vbig[:, j, 0:DH + 1],
                                 pr[:, 0:W],
                                 start=(j == 0), stop=(j == NJ - 1))
                nc.tensor.matmul(ctxB[0:DH + 1, :], vbig[:, j, 0:DH + 1],
                                 pr[:, W:2 * W],
                                 start=(j == 0), stop=(j == NJ - 1))
                # ride the next pair's QP_T matmuls in ACT's slack (the
                # j-2 shift leaves time for the pair's Wq slab to land),
                # then evacuate at j==10 so QK(t+1, 0) finds it ready.
                if t + 1 in qps_ps and 2 <= j < DMT + 2:
                    emit_qp_mm(qps_ps[t + 1], t + 1, j - 2)
                # the last pair has no QP ride, freeing one psacc bank:
                # ride (lt=2, half=0) of the output projection there
                # (he 0..5 only: he6 is normalized at (7,11), too late)
                if t == NPAIR - 1 and 1 <= j <= DMT - 2:
                    he = j - 1
                    if he == 0:
                        wc_ride[0] = psacc.tile([128, W], F32, tag="acc",
                                                name="wcr2_0")
                    nc.tensor.matmul(wc_ride[0],
                                     ctxu_sb[he][:, 256:384],
                                     wc_sb[:, he, 0:512],
                                     start=(he == 0), stop=False)
                if j == DMT + 2 and t + 1 in qps_ps:
                    qps = qps_ps.pop(t + 1)
                    sb = qpool.tile([128, LS], BF16, tag="qpt",
                                    name=f"qpt{t + 1}")
                    nc.vector.tensor_copy(sb, qps)
                    qpt_sb[t + 1] = sb
                # normalize pair t-1 in the ride-free steps j=10/11, where
                # the PE has ~450ns of slack behind ACT (at j=4/6 the bcp
                # matmuls overran the slack and rippled into the exp stream)
                if j == 10 and t > 0:
                    norm_tail_a(t - 1)
                if j == 11 and t > 0:
                    norm_tail_b(t - 1)
                if j == NJ - 1:
                    finish_pair(t)

            # ---------- out = ctx @ Wc, evacuate, store ----------
            # All (lt, half) groups accumulate he 0..6 first (overlapping
            # the last pair's normalization chain), then each group's he=7
            # lands and the result evacuates + stores, pipelined per lt.
            wc_halves = {}

            def emit_wc(lt, he_list):
                for half in range(2):
                    for he in he_list:
                        nc.tensor.matmul(
                            wc_halves[lt][half],
                            ctxu_sb[he][:, 128 * lt:128 * (lt + 1)],
                            wc_sb[:, he, 512 * half:512 * (half + 1)],
                            start=(he == 0), stop=(he == DMT - 1))

            for lt in (0, 1):
                wcp = psmm.tile([128, 1024], F32, tag="mm", name=f"wcp{lt}")
                wc_halves[lt] = (wcp[:, 0:512], wcp[:, 512:1024])
                emit_wc(lt, range(7))
            # the last pair's normalization lands here: its DVE reciprocal
            # chain overlaps the lt0/lt1 matmuls above so the bcp matmuls
            # don't stall the PE, and the psacc pool still has a free slot
            # (lt2/lt3 accumulators are allocated after).
            norm_tail(NPAIR - 1)
            # (lt=2, half=0) accumulated he 0..5 inside pair 7 (wc_ride);
            # its he=6 lands here, half=1 runs he 0..6 from scratch
            wc_halves[2] = (wc_ride[0],
                            psacc.tile([128, W], F32, tag="acc", name="wcp2_1"))
            nc.tensor.matmul(wc_halves[2][0], ctxu_sb[6][:, 256:384],
                             wc_sb[:, 6, 0:512], start=False, stop=False)
            for he in range(7):
                nc.tensor.matmul(wc_halves[2][1],
                                 ctxu_sb[he][:, 256:384],
                                 wc_sb[:, he, 512:1024],
                                 start=(he == 0), stop=False)
            wc_halves[3] = (psacc.tile([128, W], F32, tag="acc", name="wcp3_0"),
                            psacc.tile([128, W], F32, tag="acc", name="wcp3_1"))
            emit_wc(3, range(7))
            # store each 128KB half as soon as its evacuation lands (the
            # old whole-row stores waited for both halves, putting ~2us of
            # extra DMA drain on the critical tail)
            OUT_ENG = (nc.sync, nc.scalar, nc.gpsimd, nc.sync,
                       nc.gpsimd, nc.sync, nc.sync, nc.gpsimd)
            for lt in range(NLT):
                emit_wc(lt, [7])
                halves = wc_halves[lt]
                ob = mpool.tile([128, DM], BF16, tag="outsb", bufs=2)
                for half in range(2):
                    sl = slice(512 * half, 512 * (half + 1))
                    if lt % 2 == 0:
                        nc.scalar.activation(ob[:, sl], halves[half], AF.Copy)
                    else:
                        nc.vector.tensor_copy(ob[:, sl], halves[half])
                    OUT_ENG[2 * lt + half].dma_start(
                        out=out[128 * lt:128 * (lt + 1), sl], in_=ob[:, sl])

    nc.compile()
    return nc


_NC = None


def _get_nc():
    global _NC
    if _NC is None:
        _NC = build_nc()
    return _NC


def prep_in_maps(q, kv, Wq, Wkv, Wc):
    """Host-side input prep: transpose, cast to bf16, shard queries."""
    bf16 = ml_dtypes.bfloat16
    qT_full = np.ascontiguousarray(np.asarray(q, dtype=np.float32)[0].T
                                   ).astype(bf16)
    kvT = np.ascontiguousarray(np.asarray(kv, dtype=np.float32)[0].T
                               ).astype(bf16)
    # pair-major Wq: slab p = Wq[:, 128p:128(p+1)] laid out as
    # [row-in-block(128), d-block(8), col(128)], contiguous per slab
    Wq = np.asarray(Wq, dtype=np.float32).reshape(DMT, 128, NPAIR, 128)
    Wq = np.ascontiguousarray(Wq.transpose(2, 1, 0, 3).reshape(DM, DM)
                              ).astype(bf16)
    Wkv = np.ascontiguousarray(np.asarray(Wkv, dtype=np.float32)).astype(bf16)
    Wc = np.ascontiguousarray(np.asarray(Wc, dtype=np.float32)).astype(bf16)
    in_maps = []
    for i in range(N_CORES):
        in_maps.append({
            "qT": np.ascontiguousarray(qT_full[:, LS * i:LS * (i + 1)]),
            "kvT": kvT,
            "Wq": Wq,
            "Wkv": Wkv,
            "Wc": Wc,
        })
    return in_maps


def kernel(q, kv, Wq, Wkv, Wc, w):
    assert int(w) == W
    q = np.asarray(q, dtype=np.float32)
    B = q.shape[0]
    assert B == 1 and q.shape[1] == L and q.shape[2] == DM

    in_maps = prep_in_maps(q, kv, Wq, Wkv, Wc)
    nc = _get_nc()
    res = run_bass_kernel_spmd(nc, in_maps, list(range(N_CORES)))
    out = np.concatenate([res.results[i]["out"] for i in range(N_CORES)], axis=0)
    return out.reshape(1, L, DM).astype(np.float32)


# revision 70
# speedup vs baseline: 1.0045x; 1.0009x over previous
"""Trainium2 Bass kernel for LocalXLAttention (chunk-summed variant).

Math: the reference einsum sums over the chunk index z, so every query
attends to the same three [w, dh] K/V matrices built from chunk sums:
  K_prev = S_k - k_chunk[C-1], K_cur = S_k, K_next = S_k - k_chunk[0]
(and identically for V), where S_k = sum_c k_chunk[c].  Per position l
and head h:
  attn[l,h,:]  = qp[l,h,:] @ KbigT          (KbigT: [dh, 3w])
  probs        = softmax(attn, axis=-1)
  ctx[l,h,:]   = probs[l,h,:] @ Vbig        (Vbig:  [3w, dh])
  out          = ctx.reshape(L, dm) @ Wc
Sharding: L=4096 split 512 rows per core over 8 cores; each core
redundantly computes the tiny chunk-summed K/V from the full kv input.

Input loads use 1MB dma_start pieces: the ~2.5us fixed cost per
dma_start caps a ring at ~77GB/s with 256KB pieces but ~190GB/s with
1MB pieces, so the 11.25MB needed before the first QK lands in ~32us
(vs ~49us for the old 256KB layout).  kv pieces go first in d-order
(the chunk-sum tree consumes them in order), Wq/qT next, Wc during the
attention loop.

The attention loop is ACT-bound (exp at 1 elem/cycle/lane, ~1.1us per
[128,1024] step); QK pairs run concurrently on the PE's h0/h64 row
groups, PV + QP rides fill the rest of the PE slack.  Softmax
normalization stays on-chip: denominator rows -> reciprocal_approx_fast
(DVE) -> two K=1 PE matmuls broadcast 1/den across partitions -> DVE
multiply, placed a few steps after each pair so the PE never stalls on
the DVE chain.  (The previous version bounced denominators through DRAM
three times and cost ~20us of tail.)
"""

import sys
for _p in ('/opt/pypackages', '/opt/trn_rl_repo'):
    if _p not in sys.path:
        sys.path.insert(0, _p)

import numpy as np
import ml_dtypes

import concourse.bass as bass
import concourse.bacc as bacc
import concourse.tile as tile
from concourse import mybir
from concourse.bass_utils import run_bass_kernel_spmd
from concourse.masks import make_identity

F32 = mybir.dt.float32
BF16 = mybir.dt.bfloat16
AF = mybir.ActivationFunctionType

N_CORES = 8
L = 4096          # full sequence
LS = L // N_CORES # 512 rows per core
DM = 1024
NH = 16
DH = 64
W = 512           # chunk width
C = L // W        # 8 chunks
J3 = 3 * W        # 1536 softmax width
NJ = J3 // 128    # 12 j-chunks
DMT = DM // 128   # 8 dm-chunks
NPAIR = NH // 2   # 8 head pairs
NLT = LS // 128   # 4 output row chunks


def build_nc():
    nc = bacc.Bacc(None, target_bir_lowering=False)

    qT = nc.dram_tensor("qT", [DM, LS], BF16, kind="ExternalInput")
    kvT = nc.dram_tensor("kvT", [DM, L], BF16, kind="ExternalInput")
    Wq = nc.dram_tensor("Wq", [DM, DM], BF16, kind="ExternalInput")
    Wkv = nc.dram_tensor("Wkv", [DM, 2 * DH], BF16, kind="ExternalInput")
    Wc = nc.dram_tensor("Wc", [DM, DM], BF16, kind="ExternalInput")
    out = nc.dram_tensor("out", [LS, DM], BF16, kind="ExternalOutput")

    def chunk_src(dram, cols, d0, nd):
        # DRAM source AP delivering [128, nd, cols]: slot i holds rows
        # 128(d0+i):128(d0+i+1) of a [DM, cols] row-major tensor.
        base = dram[:, :]
        return bass.AP(tensor=base.tensor, offset=base.offset + d0 * 128 * cols,
                       ap=[[cols, 128], [cols * 128, nd], [1, cols]])

    with tile.TileContext(nc) as tc:
        with tc.tile_pool(name="weights", bufs=1) as wpool, \
             tc.tile_pool(name="small", bufs=1) as spool, \
             tc.tile_pool(name="qp", bufs=8) as qpool, \
             tc.tile_pool(name="stream", bufs=8) as stpool, \
             tc.tile_pool(name="kvsum", bufs=3) as kvspool, \
             tc.tile_pool(name="probs", bufs=2) as ppool, \
             tc.tile_pool(name="misc", bufs=2) as mpool, \
             tc.tile_pool(name="psacc", bufs=4, space="PSUM") as psacc, \
             tc.tile_pool(name="psmm", bufs=2, space="PSUM") as psmm:

            # ---------- input loads: 1MB pieces over the 3 DMA rings -----
            # kv first in d-order, Wq/qT next, Wc last (during the loop).
            wkv_sb = wpool.tile([128, DMT, 2 * DH], BF16, tag="wkv")
            nc.gpsimd.dma_start(out=wkv_sb, in_=chunk_src(Wkv, 2 * DH, 0, DMT))
            KV_ENG = (nc.sync, nc.scalar, nc.gpsimd, nc.sync,
                      nc.scalar, nc.gpsimd, nc.sync, nc.scalar)
            st_sb = []
            for d in range(DMT):
                st = stpool.tile([128, L], BF16, tag="kvstream", name=f"st{d}")
                st_sb.append(st)
                KV_ENG[d].dma_start(out=st,
                                    in_=kvT[128 * d:128 * (d + 1), :])
            # qT loads alongside kv; Wq arrives pair-major (host-permuted:
            # slab p holds Wq[:, 128p:128(p+1)] as [128 row-in-block, DMT,
            # 128 cols]) so only slab 0 is needed before the first QK - the
            # other slabs stream in during the attention loop, one pair
            # ahead of their QP rides.  Wc is needed only ~100us later: its
            # DMAs are emitted inside the kv-stream loop, gated on marker
            # copies (reading the last chunk sum, so the scheduler can't
            # hoist them) touching BOTH pieces' regions - the WAW dependency
            # keeps Wc out of the rings until the kv stream has drained.
            wq_sb = wpool.tile([128, NPAIR, DMT, 128], BF16, tag="wq")
            qt_sb = wpool.tile([128, DMT, LS], BF16, tag="qt")
            wc_sb = wpool.tile([128, DMT, DM], BF16, tag="wc")

            def wq_slab(p):
                base = Wq[:, :]
                return bass.AP(tensor=base.tensor,
                               offset=base.offset + p * 128 * DM,
                               ap=[[DM, 128], [1, DM]])

            # small sim-time hint: enough for the scheduler to sequence
            # these after the kv pieces in each ring (otherwise it may
            # hoist them, delaying the kv tail ~5-8us), without the
            # in-order-queue stall risk a hard semaphore gate carries
            with tc.tile_wait_until(0.01):
                nc.scalar.dma_start(out=wq_sb[:, 0, :, :], in_=wq_slab(0))
                nc.gpsimd.dma_start(out=wq_sb[:, 1, :, :], in_=wq_slab(1))
                nc.sync.dma_start(out=qt_sb, in_=chunk_src(qT, LS, 0, DMT))
            with tc.tile_wait_until(0.02):
                for p in range(2, NPAIR):
                    eng = (nc.scalar, nc.gpsimd, nc.sync)[p % 3]
                    eng.dma_start(out=wq_sb[:, p, :, :], in_=wq_slab(p))

            # ---------- constants ---------------------------------------
            zt = spool.tile([128, 512], BF16, tag="zt")
            nc.vector.memset(zt, 0.0)
            ident = spool.tile([128, 128], BF16, tag="ident")
            make_identity(nc, ident)
            o64 = spool.tile([1, DH], BF16, tag="o64")
            nc.vector.memset(o64, 1.0)
            ones_sb = spool.tile([128, 1], BF16, tag="ones")
            nc.vector.memset(ones_sb, 1.0)

            # ---------- PE warm-up during the DMA wait (HAM clock gate) --
            warm_ps = psacc.tile([128, W], F32, tag="acc", name="warm")
            for i in range(36):
                nc.tensor.matmul(warm_ps[:, 0:128], zt[:, 0:128], zt[:, 0:128],
                                 start=True, stop=True)
            # preload the exp activation table during startup
            exp_warm = spool.tile([1, 8], F32, tag="expwarm")
            nc.scalar.activation(exp_warm, zt[0:1, 0:8], AF.Exp, scale=1.0)

            # ---------- QP_T machinery (pair 0 runs mid-kv-stream) -------
            qpt_sb = [None] * NPAIR

            def emit_qp_mm(ps, t, d):
                nc.tensor.matmul(ps, wq_sb[:, t, d, :],
                                 qt_sb[:, d, :],
                                 start=(d == 0), stop=(d == DMT - 1))

            def emit_qp(t):
                ps = psacc.tile([128, W], F32, tag="acc", name=f"qps{t}")
                for d in range(DMT):
                    emit_qp_mm(ps, t, d)
                sb = qpool.tile([128, LS], BF16, tag="qpt", name=f"qpt{t}")
                nc.vector.tensor_copy(sb, ps)
                qpt_sb[t] = sb

            # ---------- kv stream: chunk-sum tree + projections ----------
            # PSUM accumulators pack K rows 0:64, V rows 64:128.
            ps0 = psacc.tile([128, W], F32, tag="acc", name="ps0")
            ps7 = psacc.tile([128, W], F32, tag="acc", name="ps7")
            pss = psacc.tile([128, W], F32, tag="acc", name="pss")
            for d in range(DMT):
                st = st_sb[d]
                nc.tensor.matmul(ps0, wkv_sb[:, d, :], st[:, 0:W],
                                 start=(d == 0), stop=(d == DMT - 1))
                nc.tensor.matmul(ps7, wkv_sb[:, d, :], st[:, L - W:L],
                                 start=(d == 0), stop=(d == DMT - 1))
                nc.vector.tensor_add(st[:, 0:2048], st[:, 0:2048], st[:, 2048:4096])
                nc.vector.tensor_add(st[:, 0:1024], st[:, 0:1024], st[:, 1024:2048])
                ks = kvspool.tile([128, W], BF16, tag="kvsum")
                nc.vector.tensor_add(ks, st[:, 0:512], st[:, 512:1024])
                nc.tensor.matmul(pss, wkv_sb[:, d, :], ks,
                                 start=(d == 0), stop=(d == DMT - 1))
                if d == 3:
                    # pair 0's QP_T runs here, in the PE's idle windows
                    # between kv piece arrivals (qT and Wq slab 0 landed
                    # long ago); its result is then ready the moment Kbig
                    # is, instead of serializing after the kv tree.
                    emit_qp(0)
                if d == DMT - 1:
                    nc.vector.tensor_copy(wc_sb[0:1, 0, 0:1], ks[0:1, 0:1])
                    nc.vector.tensor_copy(wc_sb[0:1, 4, 0:1], ks[0:1, 0:1])
                    nc.sync.dma_start(out=wc_sb[:, 0:4, :],
                                      in_=chunk_src(Wc, DM, 0, 4))
                    nc.gpsimd.dma_start(out=wc_sb[:, 4:8, :],
                                        in_=chunk_src(Wc, DM, 4, 4))

            # ---------- evacuate K/V variants to SBUF (bf16) -------------
            # interleaved with the KbigT build in j-consumption order so
            # the first QK fires ~1.2us sooner: the 'prev' block (+ its
            # row-dup, j=0..3) needs only kvs/kv7, before kv0 evacuates.
            kv0_sb = spool.tile([128, W], BF16, tag="kv0")  # K rows 0:64, V 64:128
            kv7_sb = spool.tile([128, W], BF16, tag="kv7")
            kvs_sb = spool.tile([128, W], BF16, tag="kvs")
            kbig = spool.tile([128, J3], BF16, tag="kbig")
            # KbigT [128, 1536] = [prev | cur | next]; rows 64:128
            # duplicate rows 0:64 so the QK pair can run on both PE row
            # groups concurrently.
            nc.vector.tensor_copy(kvs_sb, pss)
            nc.vector.tensor_copy(kv7_sb, ps7)
            nc.vector.tensor_sub(kbig[0:DH, 0:W], kvs_sb[0:DH, :], kv7_sb[0:DH, :])
            nc.vector.tensor_copy(kbig[DH:2 * DH, 0:W], kbig[0:DH, 0:W])
            nc.vector.tensor_copy(kbig[0:DH, W:2 * W], kvs_sb[0:DH, :])
            nc.vector.tensor_copy(kbig[DH:2 * DH, W:2 * W], kvs_sb[0:DH, :])
            nc.vector.tensor_copy(kv0_sb, ps0)
            nc.vector.tensor_sub(kbig[0:DH, 2 * W:3 * W], kvs_sb[0:DH, :],
                                 kv0_sb[0:DH, :])
            nc.vector.tensor_copy(kbig[DH:2 * DH, 2 * W:3 * W],
                                  kbig[0:DH, 2 * W:3 * W])

            # ---------- Vbig [128, 12, 65(+pad)] -------------------------
            vbig = spool.tile([128, NJ, 68], BF16, tag="vbig")
            for j in range(NJ):
                nc.vector.tensor_copy(vbig[:, j, DH:DH + 1], ones_sb)
            for yt in range(4):
                tps = psacc.tile([128, DH], BF16, tag="acc")
                tp0 = psacc.tile([128, DH], BF16, tag="acc")
                tp7 = psacc.tile([128, DH], BF16, tag="acc")
                sl = slice(128 * yt, 128 * (yt + 1))
                # V rows live at base partition 64; ident[64:128, 64:128]
                # is an identity block at the matching base.
                idq = ident[DH:128, DH:128]
                nc.tensor.transpose(tps, kvs_sb[DH:128, sl], idq)
                nc.tensor.transpose(tp0, kv0_sb[DH:128, sl], idq)
                nc.tensor.transpose(tp7, kv7_sb[DH:128, sl], idq)
                nc.vector.tensor_copy(vbig[:, 4 + yt, 0:DH], tps)
                nc.vector.tensor_sub(vbig[:, 0 + yt, 0:DH], vbig[:, 4 + yt, 0:DH], tp7)
                nc.vector.tensor_sub(vbig[:, 8 + yt, 0:DH], vbig[:, 4 + yt, 0:DH], tp0)

            # a few warm matmuls keep the PE clock ramped through the DVE
            # kbig/vbig construction so the first QKs don't run cold (a
            # fresh psmm tile: warm_ps's psacc slot was recycled long ago)
            warm2 = psmm.tile([128, 1024], F32, tag="mm", name="warm2")
            for i in range(10):
                nc.tensor.matmul(warm2[:, 0:128], zt[:, 0:128], zt[:, 0:128],
                                 start=True, stop=True)

            # ---------- attention: QK -> exp(PSUM direct) -> PV ----------
            ctxu_sb = []  # per pair [128, 512]: rows 0:64 head 2t, 64:128 head 2t+1
            for t in range(NPAIR):
                ctxu_sb.append(qpool.tile([128, W], BF16, tag="ctxu",
                                          name=f"ctxu{t}"))

            steps = [(t, j) for t in range(NPAIR) for j in range(NJ)]
            ctx_ps = {}   # t -> (ctxA, ctxB)
            qps_ps = {}   # t -> psum tile being accumulated
            qk_tiles = {}
            rdb_sb = {}   # t -> [1, 2W] bf16 reciprocal denominators
            wc_ride = {}  # psum tile for the (lt=2, half=0) Wc ride

            def emit_qk(t, j):
                qpt = qpt_sb[t]
                qk = psmm.tile([128, 1024], F32, tag="mm", name=f"qk{t}_{j}")
                nc.tensor.matmul(qk[:, 0:W],
                                 kbig[0:DH, 128 * j:128 * (j + 1)],
                                 qpt[0:DH, :], start=True, stop=True)
                nc.tensor.matmul(qk[:, W:2 * W],
                                 kbig[DH:2 * DH, 128 * j:128 * (j + 1)],
                                 qpt[DH:128, :], start=True, stop=True)
                qk_tiles[(t, j)] = qk

            def finish_pair(t):
                # Evacuate unnormalized context + the denominator rows,
                # then build 1/den while the next pair streams.  The PE
                # broadcast + DVE multiply run a few steps later
                # (norm_tail) so the PE never waits on this DVE chain.
                ctxA, ctxB = ctx_ps.pop(t)
                cu = ctxu_sb[t]
                dd = mpool.tile([1, 2 * W], F32, tag="dd", name=f"dd{t}", bufs=2)
                nc.vector.tensor_copy(cu[0:DH, :], ctxA[0:DH, :])
                nc.vector.tensor_copy(dd[0:1, 0:W], ctxA[DH:DH + 1, :])
                nc.vector.tensor_copy(cu[DH:128, :], ctxB[0:DH, :])
                nc.vector.tensor_copy(dd[0:1, W:2 * W], ctxB[DH:DH + 1, :])
                rd = mpool.tile([1, 2 * W], F32, tag="rd", name=f"rd{t}", bufs=2)
                nc.vector.reciprocal_approx_fast(out=rd, in_=dd)
                rdb = mpool.tile([1, 2 * W], BF16, tag="rdb", name=f"rdb{t}",
                                 bufs=2)
                nc.vector.tensor_copy(rdb, rd)
                rdb_sb[t] = rdb

            bcp_ps = {}

            def norm_tail_a(t):
                # broadcast 1/den across partitions with two K=1 PE matmuls
                # (rows 0:64 <- head 2t, rows 64:128 <- head 2t+1), then
                # scale ctxu in place.  Split in two halves (called at
                # different steps) so the inserted PE work never exceeds the
                # per-step slack behind ACT.
                bcp = psacc.tile([128, W], F32, tag="acc", name=f"bcp{t}")
                bcp_ps[t] = bcp
                nc.tensor.matmul(bcp[0:DH, :], o64, rdb_sb[t][0:1, 0:W],
                                 start=True, stop=True)
                nc.vector.tensor_mul(ctxu_sb[t][0:DH, :],
                                     ctxu_sb[t][0:DH, :], bcp[0:DH, :])

            def norm_tail_b(t):
                rdb = rdb_sb.pop(t)
                bcp = bcp_ps.pop(t)
                nc.tensor.matmul(bcp[DH:128, :], o64, rdb[0:1, W:2 * W],
                                 start=True, stop=True)
                nc.vector.tensor_mul(ctxu_sb[t][DH:128, :],
                                     ctxu_sb[t][DH:128, :], bcp[DH:128, :])

            def norm_tail(t):
                norm_tail_a(t)
                norm_tail_b(t)

            emit_qk(*steps[0])
            for s in range(len(steps)):
                t, j = steps[s]
                if s + 1 < len(steps):
                    emit_qk(*steps[s + 1])
                if j == 0:
                    ctxA = psacc.tile([128, W], F32, tag="acc", name=f"ctxA{t}")
                    ctxB = psacc.tile([128, W], F32, tag="acc", name=f"ctxB{t}")
                    ctx_ps[t] = (ctxA, ctxB)
                    if t + 1 < NPAIR:
                        qps_ps[t + 1] = psacc.tile([128, W], F32, tag="acc",
                                                   name=f"qps{t + 1}")
                ctxA, ctxB = ctx_ps[t]
                qk = qk_tiles.pop((t, j))
                pr = ppool.tile([128, 1024], BF16, tag="probs",
                                name=f"pr{t}_{j}")
                nc.scalar.activation(pr, qk, AF.Exp, scale=0.125)
                nc.tensor.matmul(ctxA[0:DH + 1, :], vbig[:, j, 0:DH + 1],
                                 pr[:, 0:W],
                                 start=(j == 0), stop=(j == NJ - 1))
                nc.tensor.matmul(ctxB[0:DH + 1, :], vbig[:, j, 0:DH + 1],
                                 pr[:, W:2 * W],
                                 start=(j == 0), stop=(j == NJ - 1))
                # ride the next pair's QP_T matmuls in ACT's slack (the
                # j-2 shift leaves time for the pair's Wq slab to land),
                # then evacuate at j==10 so QK(t+1, 0) finds it ready.
                if t + 1 in qps_ps and 2 <= j < DMT + 2:
                    emit_qp_mm(qps_ps[t + 1], t + 1, j - 2)
                # the last pair has no QP ride, freeing one psacc bank:
                # ride (lt=2, half=0) of the output projection there
                # (he 0..5 only: he6 is normalized at (7,11), too late)
                if t == NPAIR - 1 and 1 <= j <= DMT - 2:
                    he = j - 1
                    if he == 0:
                        wc_ride[0] = psacc.tile([128, W], F32, tag="acc",
                                                name="wcr2_0")
                    nc.tensor.matmul(wc_ride[0],
                                     ctxu_sb[he][:, 256:384],
                                     wc_sb[:, he, 0:512],
                                     start=(he == 0), stop=False)
                if j == DMT + 2 and t + 1 in qps_ps:
                    qps = qps_ps.pop(t + 1)
                    sb = qpool.tile([128, LS], BF16, tag="qpt",
                                    name=f"qpt{t + 1}")
                    nc.vector.tensor_copy(sb, qps)
                    qpt_sb[t + 1] = sb
                # normalize pair t-1 in the ride-free steps j=10/11, where
                # the PE has ~450ns of slack behind ACT (at j=4/6 the bcp
                # matmuls overran the slack and rippled into the exp stream)
                if j == 10 and t > 0:
                    norm_tail_a(t - 1)
                if j == 11 and t > 0:
                    norm_tail_b(t - 1)
                if j == NJ - 1:
                    finish_pair(t)

            # ---------- out = ctx @ Wc, evacuate, store ----------
            # All (lt, half) groups accumulate he 0..6 first (overlapping
            # the last pair's normalization chain), then each group's he=7
            # lands and the result evacuates + stores, pipelined per lt.
            wc_halves = {}

            def emit_wc(lt, he_list):
                for half in range(2):
                    for he in he_list:
                        nc.tensor.matmul(
                            wc_halves[lt][half],
                            ctxu_sb[he][:, 128 * lt:128 * (lt + 1)],
                            wc_sb[:, he, 512 * half:512 * (half + 1)],
                            start=(he == 0), stop=(he == DMT - 1))

            for lt in (0, 1):
                wcp = psmm.tile([128, 1024], F32, tag="mm", name=f"wcp{lt}")
                wc_halves[lt] = (wcp[:, 0:512], wcp[:, 512:1024])
                emit_wc(lt, range(7))
            # the last pair's normalization lands here: its DVE reciprocal
            # chain overlaps the lt0/lt1 matmuls above so the bcp matmuls
            # don't stall the PE, and the psacc pool still has a free slot
            # (lt2/lt3 accumulators are allocated after).
            norm_tail(NPAIR - 1)
            # (lt=2, half=0) accumulated he 0..5 inside pair 7 (wc_ride);
            # its he=6 lands here, half=1 runs he 0..6 from scratch
            wc_halves[2] = (wc_ride[0],
                            psacc.tile([128, W], F32, tag="acc", name="wcp2_1"))
            nc.tensor.matmul(wc_halves[2][0], ctxu_sb[6][:, 256:384],
                             wc_sb[:, 6, 0:512], start=False, stop=False)
            for he in range(7):
                nc.tensor.matmul(wc_halves[2][1],
                                 ctxu_sb[he][:, 256:384],
                                 wc_sb[:, he, 512:1024],
                                 start=(he == 0), stop=False)
            wc_halves[3] = (psacc.tile([128, W], F32, tag="acc", name="wcp3_0"),
                            psacc.tile([128, W], F32, tag="acc", name="wcp3_1"))
            emit_wc(3, range(7))
            # store each 128KB half as soon as its evacuation lands (the
            # old whole-row stores waited for both halves, putting ~2us of
            # extra DMA drain on the critical tail)
            OUT_ENG = (nc.sync, nc.scalar, nc.gpsimd, nc.sync,
                       nc.gpsimd, nc.sync, nc.sync, nc.gpsimd)
            for lt in range(NLT):
                emit_wc(lt, [7])
                halves = wc_halves[lt]
                ob = mpool.tile([128, DM], BF16, tag="outsb", bufs=2)
                for half in range(2):
                    sl = slice(512 * half, 512 * (half + 1))
                    if lt % 2 == 0:
                        nc.scalar.activation(ob[:, sl], halves[half], AF.Copy)
                    else:
                        nc.vector.tensor_copy(ob[:, sl], halves[half])
                    OUT_ENG[2 * lt + half].dma_start(
                        out=out[128 * lt:128 * (lt + 1), sl], in_=ob[:, sl])

    nc.compile()
    return nc


_NC = None


def _get_nc():
    global _NC
    if _NC is None:
        _NC = build_nc()
    return _NC


def prep_in_maps(q, kv, Wq, Wkv, Wc):
    """Host-side input prep: transpose, cast to bf16, shard queries."""
    bf16 = ml_dtypes.bfloat16
    qT_full = np.ascontiguousarray(np.asarray(q, dtype=np.float32)[0].T
                                   ).astype(bf16)
    kvT = np.ascontiguousarray(np.asarray(kv, dtype=np.float32)[0].T
                               ).astype(bf16)
    # pair-major Wq: slab p = Wq[:, 128p:128(p+1)] laid out as
    # [row-in-block(128), d-block(8), col(128)], contiguous per slab
    Wq = np.asarray(Wq, dtype=np.float32).reshape(DMT, 128, NPAIR, 128)
    Wq = np.ascontiguousarray(Wq.transpose(2, 1, 0, 3).reshape(DM, DM)
                              ).astype(bf16)
    Wkv = np.ascontiguousarray(np.asarray(Wkv, dtype=np.float32)).astype(bf16)
    Wc = np.ascontiguousarray(np.asarray(Wc, dtype=np.float32)).astype(bf16)
    in_maps = []
    for i in range(N_CORES):
        in_maps.append({
            "qT": np.ascontiguousarray(qT_full[:, LS * i:LS * (i + 1)]),
            "kvT": kvT,
            "Wq": Wq,
            "Wkv": Wkv,
            "Wc": Wc,
        })
    return in_maps


def kernel(q, kv, Wq, Wkv, Wc, w):
    assert int(w) == W
    q = np.asarray(q, dtype=np.float32)
    B = q.shape[0]
    assert B == 1 and q.shape[1] == L and q.shape[2] == DM

    in_maps = prep_in_maps(q, kv, Wq, Wkv, Wc)
    nc = _get_nc()
    res = run_bass_kernel_spmd(nc, in_maps, list(range(N_CORES)))
    out = np.concatenate([res.results[i]["out"] for i in range(N_CORES)], axis=0)
    return out.reshape(1, L, DM).astype(np.float32)
